# revision 26
# baseline (speedup 1.0000x reference)
"""Child-Sum TreeLSTM (perfect binary tree, depth 14) on 8 Trainium2 NeuronCores.

Single-stage SPMD design (v2):
- Heap-order contiguous node sharding: core k owns nodes [k*n/8, (k+1)*n/8)
  of every level lvl >= 3; levels 13..3 are communication-free.
- Levels 13..8 (98.5% of work) run in fp8(e4m3) DoubleRow matmuls (K=256 per
  instruction): weights/x/h are stored as 16*value in fp8 (avoids e4m3
  subnormals); pre-activations accumulate 256*true in PSUM f32 and the
  activation applies scale=1/256. The x-projection is folded into each
  level's GEMM as 2 extra DR K-chunks, with the bias folded in via a
  const-1.0 row in the second x chunk.
- Levels 7..3 and the top levels 2..0 run in bf16 (numerics: fp8 errors at
  the top of the tree hit the root directly; bf16 there keeps rel_err ~1e-2).
- After level 3 an 8-core DRAM AllGather shares the 8 (h,c) level-3 states;
  every core then computes levels 2..0 redundantly and writes the root.
- State layout: [128 partitions (mem rows of K-chunk), 8 K-chunks * n nodes];
  h stored fp8 (16*h) at fp8 levels, bf16 at bf16 levels; c always bf16.
"""

import numpy as np
import ml_dtypes
from contextlib import ExitStack

import concourse.bass as bass
import concourse.tile as tile
from concourse import bacc, mybir
from concourse.bass_utils import run_bass_kernel_spmd
from concourse.alu_op_type import AluOpType

NPF8 = ml_dtypes.float8_e4m3fn
NPBF = ml_dtypes.bfloat16
F32 = mybir.dt.float32
BF = mybir.dt.bfloat16
FP8 = mybir.dt.float8e4
DRM = mybir.MatmulPerfMode.DoubleRow
SIG = mybir.ActivationFunctionType.Sigmoid
TANH = mybir.ActivationFunctionType.Tanh

P = 128
MEM = 1024
MT = 8
IN = 300
DEPTH = 14
NCORE = 8
S = 16.0
INV_SS = 1.0 / (S * S)
GATES = "ifou"

FP8LVLS = [13, 12, 11, 10, 9, 8]
BFLVLS = [7, 6, 5, 4, 3]
NC8 = 2016
OFF8 = {13: 0, 12: 1024, 11: 1536, 10: 1792, 9: 1920, 8: 1984}
NCB = 38
OFFB = {7: 0, 6: 16, 5: 24, 4: 28, 3: 30}
TOFF = {2: 31, 1: 35, 0: 37}
PKX = [128, 128, 45]  # bf16 x-chunk partition counts (chunk2 row 44 = const 1)


def _r2(ap):
    return ap.rearrange("p (two c) -> p two c", two=2)


def _segments(c0, length, n):
    """Split state-col range [c0, c0+length) into (m, s0, sl) per-m pieces."""
    out = []
    cur = c0
    end = c0 + length
    while cur < end:
        m = cur // n
        s0 = cur % n
        sl = min(n - s0, end - cur)
        out.append((m, s0, sl))
        cur += sl
    return out


def _emit_fp8_level(nc, W, pools, h_ch, c_ch, n, lvl, out_fp8):
    """One fp8-DR level. h_ch [128, 8*2n] fp8(16h), c_ch [128, 8*2n] bf16.
    Leaf level: h_ch is None. Returns (h_new, c_new).
    Processes the [8*n]-col state space in blocks of <=512 cols."""
    gp, hp, psum = pools
    wh8_ap, wx8_ap, wxb8_ap, xa3, xb3, bslot = W
    off = OFF8[lvl]
    n2 = 2 * n
    NT = MT * n
    V, G = nc.vector, nc.gpsimd
    par = "o" if lvl & 1 else "e"
    gi_f = GATES.index("f")

    h_new = hp.tile([P, NT], FP8 if out_fp8 else BF, tag=f"h_{par}", name=f"h{lvl}")
    c_new = hp.tile([P, NT], BF, tag=f"c_{par}", name=f"c{lvl}")

    if h_ch is not None:
        hc3 = h_ch[:].rearrange("p (k c) -> p k c", k=MT)
        # x-dup for the f gate: parent col j -> child cols 2j, 2j+1
        xda = gp.tile([P, 2 * n2], FP8, tag="xda", bufs=1, name=f"xda{lvl}")
        xdb = gp.tile([P, 2 * n2], FP8, tag="xdb", bufs=1, name=f"xdb{lvl}")
        for b in range(2):
            V.tensor_copy(_r2(xda[:])[:, :, b:n2:2], xa3[:, :, off : off + n])
            G.tensor_copy(_r2(xdb[:])[:, :, b:n2:2], xb3[:, :, off : off + n])
        xda3, xdb3 = _r2(xda[:]), _r2(xdb[:])

        # hsum in fp8 (units of 16*h): split across both vector engines
        hs = hp.tile([P, NT], FP8, tag=f"hs_{par}", name=f"hs{lvl}")
        hs3 = hs[:].rearrange("p (k c) -> p k c", k=MT)
        he = hc3[:, :, 0:n2:2]
        ho = hc3[:, :, 1:n2:2]
        V.tensor_add(hs3[:, 0:4], he[:, 0:4], ho[:, 0:4])
        G.tensor_add(hs3[:, 4:8], he[:, 4:8], ho[:, 4:8])

        # f gate + fc, blockwise over child-col space; fc lands in c_new
        for C0 in range(0, NT, 512):
            BS = min(512, NT - C0)
            gf = gp.tile([P, 2 * BS], BF, tag="gf", bufs=2, name=f"gf{lvl}_{C0}")
            for t0 in range(0, 2 * BS, 512):
                tl = min(512, 2 * BS - t0)
                ps = psum.tile([P, tl], F32, tag="ps", bufs=8, name=f"pf{lvl}_{C0}_{t0}")
                for m, s0, sl in _segments(2 * C0 + t0, tl, n2):
                    d0 = m * n2 + s0 - (2 * C0 + t0)
                    for k2 in range(4):
                        nc.tensor.matmul(
                            ps[:, d0 : d0 + sl], wh8_ap(gi_f, k2, m),
                            hc3[:, 2 * k2 : 2 * k2 + 2, s0 : s0 + sl],
                            start=(k2 == 0), stop=False, perf_mode=DRM)
                    nc.tensor.matmul(ps[:, d0 : d0 + sl], wx8_ap(gi_f, m),
                                     xda3[:, :, s0 : s0 + sl],
                                     start=False, stop=False, perf_mode=DRM)
                    p0 = bslot(gi_f, m)
                    nc.tensor.matmul(ps[:, d0 : d0 + sl], wxb8_ap(gi_f, m),
                                     xdb3[p0 : p0 + 23, :, s0 : s0 + sl],
                                     start=False, stop=True, perf_mode=DRM)
                nc.scalar.activation(gf[:, t0 : t0 + tl], ps[:], SIG, scale=INV_SS)
            prod = gp.tile([P, 2 * BS], BF, tag="pr", bufs=2, name=f"pr{lvl}_{C0}")
            V.tensor_mul(prod[:], gf[:], c_ch[:, 2 * C0 : 2 * C0 + 2 * BS])
            V.tensor_add(c_new[:, C0 : C0 + BS],
                         prod[:, 0 : 2 * BS : 2], prod[:, 1 : 2 * BS : 2])
    else:
        hs3 = None

    # i/o/u gates, blockwise; then state math per block
    gio = {}
    for C0 in range(0, NT, 512):
        BS = min(512, NT - C0)
        segs = _segments(C0, BS, n)
        for g in "iou":
            gi = GATES.index(g)
            gt = gp.tile([P, BS], BF, tag=f"g{g}", bufs=3, name=f"g{g}{lvl}_{C0}")
            ps = psum.tile([P, BS], F32, tag="ps", bufs=8, name=f"p{g}{lvl}_{C0}")
            for m, s0, sl in segs:
                d0 = m * n + s0 - C0
                first = True
                if hs3 is not None:
                    for k2 in range(4):
                        nc.tensor.matmul(
                            ps[:, d0 : d0 + sl], wh8_ap(gi, k2, m),
                            hs3[:, 2 * k2 : 2 * k2 + 2, s0 : s0 + sl],
                            start=first, stop=False, perf_mode=DRM)
                        first = False
                nc.tensor.matmul(ps[:, d0 : d0 + sl], wx8_ap(gi, m),
                                 xa3[:, :, off + s0 : off + s0 + sl],
                                 start=first, stop=False, perf_mode=DRM)
                p0 = bslot(gi, m)
                nc.tensor.matmul(ps[:, d0 : d0 + sl], wxb8_ap(gi, m),
                                 xb3[p0 : p0 + 23, :, off + s0 : off + s0 + sl],
                                 start=False, stop=True, perf_mode=DRM)
            nc.scalar.activation(gt[:], ps[:], TANH if g == "u" else SIG, scale=INV_SS)
            gio[g] = gt
        blk = slice(C0, C0 + BS)
        if h_ch is not None:
            iu = gp.tile([P, BS], BF, tag="iu", bufs=2, name=f"iu{lvl}_{C0}")
            G.tensor_mul(iu[:], gio["i"][:], gio["u"][:])
            V.tensor_add(c_new[:, blk], c_new[:, blk], iu[:])
        else:
            G.tensor_mul(c_new[:, blk], gio["i"][:], gio["u"][:])
        th = gp.tile([P, BS], BF, tag="th", bufs=2, name=f"th{lvl}_{C0}")
        nc.scalar.activation(th[:], c_new[:, blk], TANH)
        if out_fp8:
            V.scalar_tensor_tensor(h_new[:, blk], gio["o"][:], S, th[:],
                                   AluOpType.mult, AluOpType.mult)
        else:
            G.tensor_mul(h_new[:, blk], gio["o"][:], th[:])
    return h_new, c_new


def _emit_bf16_level(nc, W, pools, h_ch, c_ch, n, xoff, lvl):
    """One bf16 level (n <= 16). h_ch/c_ch [128, 8*2n] bf16."""
    gp, hp, psum = pools
    whb_ap, wxb_ap, xtb3, b2slot = W
    n2 = 2 * n
    V, G = nc.vector, nc.gpsimd
    par = "o" if lvl & 1 else "e"

    h_new = hp.tile([P, MT * n], BF, tag=f"hb_{par}", name=f"hb{lvl}")
    c_new = hp.tile([P, MT * n], BF, tag=f"cb_{par}", name=f"cb{lvl}")

    hc3 = h_ch[:].rearrange("p (k c) -> p k c", k=MT)
    # x-dup for f
    xd = gp.tile([P, 3 * n2], BF, tag="xdc", bufs=3, name=f"xd{lvl}")
    xd3 = xd[:].rearrange("p (kx c) -> p kx c", kx=3)
    for b in range(2):
        V.tensor_copy(xd3[:, :, b:n2:2], xtb3[:, :, xoff : xoff + n])

    # f gate first
    gi_f = GATES.index("f")
    psf = psum.tile([P, MT * n2], F32, tag="ps", bufs=8, name=f"pfb{lvl}")
    for m in range(MT):
        c0 = m * n2
        for k in range(MT):
            nc.tensor.matmul(psf[:, c0 : c0 + n2], whb_ap(gi_f, k, m),
                             hc3[:, k], start=(k == 0), stop=False)
        for kx in range(2):
            nc.tensor.matmul(psf[:, c0 : c0 + n2], wxb_ap(gi_f, m, kx),
                             xd3[:, kx], start=False, stop=False)
        p0 = b2slot(gi_f, m)
        nc.tensor.matmul(psf[:, c0 : c0 + n2], wxb_ap(gi_f, m, 2),
                         xd3[p0 : p0 + 45, 2], start=False, stop=True)
    gf = gp.tile([P, MT * n2], BF, tag="gfa", bufs=2, name=f"gfb{lvl}")
    nc.scalar.activation(gf[:], psf[:], SIG)

    # hsum
    hs = hp.tile([P, MT * n], BF, tag=f"hsb_{par}", name=f"hsb{lvl}")
    hs3 = hs[:].rearrange("p (k c) -> p k c", k=MT)
    V.tensor_add(hs3, hc3[:, :, 0:n2:2], hc3[:, :, 1:n2:2])

    # i/o/u in one psum tile: gate gidx at cols gidx*8n
    ps3 = psum.tile([P, 3 * MT * n], F32, tag="ps", bufs=8, name=f"piou{lvl}")
    for gidx, g in enumerate("iou"):
        gi = GATES.index(g)
        for m in range(MT):
            c0 = gidx * MT * n + m * n
            for k in range(MT):
                nc.tensor.matmul(ps3[:, c0 : c0 + n], whb_ap(gi, k, m),
                                 hs3[:, k], start=(k == 0), stop=False)
            for kx in range(2):
                nc.tensor.matmul(ps3[:, c0 : c0 + n], wxb_ap(gi, m, kx),
                                 xtb3[:, kx, xoff : xoff + n],
                                 start=False, stop=False)
            p0 = b2slot(gi, m)
            nc.tensor.matmul(ps3[:, c0 : c0 + n], wxb_ap(gi, m, 2),
                             xtb3[p0 : p0 + 45, 2, xoff : xoff + n],
                             start=False, stop=True)
    giou = gp.tile([P, 3 * MT * n], BF, tag="giou", bufs=2, name=f"giou{lvl}")
    nc.scalar.activation(giou[:, 0 : 2 * MT * n], ps3[:, 0 : 2 * MT * n], SIG)
    nc.scalar.activation(giou[:, 2 * MT * n :], ps3[:, 2 * MT * n :], TANH)

    # state math
    prod = gp.tile([P, MT * n2], BF, tag="pr", bufs=2, name=f"prb{lvl}")
    pr3 = prod[:].rearrange("p (k c) -> p k c", k=MT)
    V.tensor_mul(prod[:], gf[:], c_ch[:])
    cc3 = c_new[:].rearrange("p (k c) -> p k c", k=MT)
    V.tensor_add(cc3, pr3[:, :, 0:n2:2], pr3[:, :, 1:n2:2])
    iu = gp.tile([P, MT * n], BF, tag="iu", bufs=2, name=f"iub{lvl}")
    G.tensor_mul(iu[:], giou[:, 0 : MT * n], giou[:, 2 * MT * n :])
    V.tensor_add(c_new[:], c_new[:], iu[:])
    th = gp.tile([P, MT * n], BF, tag="th", bufs=2, name=f"thb{lvl}")
    nc.scalar.activation(th[:], c_new[:], TANH)
    G.tensor_mul(h_new[:], giou[:, MT * n : 2 * MT * n], th[:])
    return h_new, c_new


def build():
    nc = bacc.Bacc("TRN2", target_bir_lowering=False, debug=False, num_devices=NCORE)
    xt8a_d = nc.dram_tensor("xt8a", [P, 2 * NC8], FP8, kind="ExternalInput").ap()
    xt8b_d = nc.dram_tensor("xt8b", [P, 2 * NC8], FP8, kind="ExternalInput").ap()
    xtb_d = nc.dram_tensor("xtb", [P, 3 * NCB], BF, kind="ExternalInput").ap()
    wx8_d = nc.dram_tensor("wx8", [P, 4 * MT * 256], FP8, kind="ExternalInput").ap()
    wxb8_d = nc.dram_tensor("wxb8", [P, 11 * 256], FP8, kind="ExternalInput").ap()
    wh8_d = nc.dram_tensor("wh8", [P, 4 * 4 * MT * 256], FP8, kind="ExternalInput").ap()
    whb_d = nc.dram_tensor("whb", [P, 4 * MT * MEM], BF, kind="ExternalInput").ap()
    wxb_d = nc.dram_tensor("wxb", [P, 2 * 32 * P + 16 * P], BF, kind="ExternalInput").ap()
    root_d = nc.dram_tensor("root", [P, 16], F32, kind="ExternalOutput").ap()
    cc_in = nc.dram_tensor("cc_in", [P, 16], F32, kind="Internal").ap()
    cc_out = nc.dram_tensor("cc_out", [NCORE, P, 16], F32, kind="Internal",
                            addr_space="Shared").ap()

    with tile.TileContext(nc) as tc, ExitStack() as ctx:
        const = ctx.enter_context(tc.tile_pool(name="const", bufs=1))
        gp = ctx.enter_context(tc.tile_pool(name="gp", bufs=4))
        hp = ctx.enter_context(tc.tile_pool(name="hp", bufs=1))
        psum = ctx.enter_context(tc.tile_pool(name="psum", bufs=8, space="PSUM"))
        pools = (gp, hp, psum)

        # loads in need-order: leaf deps first, whb/wxb/xtb last
        xt8a = const.tile([P, 2 * NC8], FP8, name="xt8a")
        nc.sync.dma_start(xt8a[:], xt8a_d[:])
        xt8b = const.tile([P, 2 * NC8], FP8, name="xt8b")
        nc.sync.dma_start(xt8b[:], xt8b_d[:])
        wx8 = const.tile([P, 4 * MT * 256], FP8, name="wx8")
        nc.sync.dma_start(wx8[:], wx8_d[:])
        wxb8 = const.tile([P, 11 * 256], FP8, name="wxb8")
        nc.sync.dma_start(wxb8[:], wxb8_d[:])
        wh8 = const.tile([P, 4 * 4 * MT * 256], FP8, name="wh8")
        for q in range(4):
            sl = slice(q * 4 * MT * 256, (q + 1) * 4 * MT * 256)
            nc.sync.dma_start(wh8[:, sl], wh8_d[:, sl])
        whb = const.tile([P, 4 * MT * MEM], BF, name="whb")
        for q in range(4):
            sl = slice(q * MT * MEM, (q + 1) * MT * MEM)
            nc.sync.dma_start(whb[:, sl], whb_d[:, sl])
        wxb = const.tile([P, 2 * 32 * P + 16 * P], BF, name="wxb")
        nc.sync.dma_start(wxb[:], wxb_d[:])
        xtb = const.tile([P, 3 * NCB], BF, name="xtb")
        nc.sync.dma_start(xtb[:], xtb_d[:])

        xa3 = _r2(xt8a[:])
        xb3 = _r2(xt8b[:])
        xtb3 = xtb[:].rearrange("p (kx c) -> p kx c", kx=3)

        def wh8_ap(gi, k2, m):
            b = ((gi * 4 + k2) * MT + m) * 256
            return _r2(wh8[:, b : b + 256])

        def wx8_ap(gi, m):
            b = (gi * MT + m) * 256
            return _r2(wx8[:, b : b + 256])

        def bslot(gi, m):
            return 32 * ((gi * MT + m) % 3)

        def wxb8_ap(gi, m):
            # B-chunks (23 rows) packed 4-per-partition-column at 32-alignment
            j = gi * MT + m
            p0 = 32 * (j % 3)
            b = (j // 3) * 256
            return _r2(wxb8[:, b : b + 256])[p0 : p0 + 23]

        def whb_ap(gi, k, m):
            b = (gi * MT + k) * MEM + m * P
            return whb[:, b : b + P]

        def wxb_ap(gi, m, kx):
            j = gi * MT + m
            if kx < 2:
                return wxb[:, (j * 2 + kx) * P : (j * 2 + kx + 1) * P]
            # kx2 chunks (45 rows) packed 2-per-column at 64-alignment
            p0 = 64 * (j % 2)
            b = 2 * 32 * P + (j // 2) * P
            return wxb[p0 : p0 + 45, b : b + P]

        def b2slot(gi, m):
            return 64 * ((gi * MT + m) % 2)

        W8 = (wh8_ap, wx8_ap, wxb8_ap, xa3, xb3, bslot)
        WB = (whb_ap, wxb_ap, xtb3, b2slot)

        h_ch = c_ch = None
        for lvl in FP8LVLS:
            n = (1 << lvl) // NCORE
            h_ch, c_ch = _emit_fp8_level(nc, W8, pools, h_ch, c_ch, n, lvl,
                                         out_fp8=(lvl != 8))
        for lvl in BFLVLS:
            n = (1 << lvl) // NCORE
            h_ch, c_ch = _emit_bf16_level(nc, WB, pools, h_ch, c_ch, n, OFFB[lvl], lvl)

        # AllGather the 8 level-3 (h, c) states
        out32 = gp.tile([P, 16], F32, tag="o32", bufs=1, name="out32")
        nc.vector.tensor_copy(out32[:, 0:8], h_ch[:])
        nc.vector.tensor_copy(out32[:, 8:16], c_ch[:])
        nc.sync.dma_start(cc_in[:], out32[:])
        nc.gpsimd.collective_compute(
            "AllGather", mybir.AluOpType.bypass,
            replica_groups=[list(range(NCORE))],
            ins=[cc_in[:]], outs=[cc_out[:]])
        ght = gp.tile([P, NCORE * 16], F32, tag="ght", bufs=1, name="ght")
        for j in range(NCORE):
            nc.sync.dma_start(ght[:, j * 16 : (j + 1) * 16], cc_out[j])
        ghp = ght[:].rearrange("p (j c) -> p c j", j=NCORE)  # [128, 16, 8]
        h_ch = hp.tile([P, 64], BF, tag="hb_o", name="htop")
        c_ch = hp.tile([P, 64], BF, tag="cb_o", name="ctop")
        nc.vector.tensor_copy(h_ch[:].rearrange("p (k j) -> p k j", k=8), ghp[:, 0:8])
        nc.vector.tensor_copy(c_ch[:].rearrange("p (k j) -> p k j", k=8), ghp[:, 8:16])

        for lvl in (2, 1, 0):
            n = 1 << lvl
            h_ch, c_ch = _emit_bf16_level(nc, WB, pools, h_ch, c_ch, n, TOFF[lvl], lvl)

        outr = gp.tile([P, 16], F32, tag="o32", bufs=1, name="outr")
        nc.vector.tensor_copy(outr[:, 0:8], c_ch[:])
        nc.vector.tensor_copy(outr[:, 8:16], h_ch[:])
        nc.sync.dma_start(root_d[:], outr[:])
    nc.compile()
    return nc


_CACHE = {}


def _get_program():
    if "p" not in _CACHE:
        _CACHE["p"] = build()
    return _CACHE["p"]


def _prep_host_inputs(embs, Ws, bs):
    f32 = np.float32
    # fp8 weights (16*W)
    wh8 = np.zeros((P, 4 * 4 * MT * 256), NPF8)
    wx8 = np.zeros((P, 4 * MT * 256), NPF8)
    wxb8 = np.zeros((P, 11 * 256), NPF8)
    whb = np.zeros((P, 4 * MT * MEM), NPBF)
    wxb = np.zeros((P, 2 * 32 * P + 16 * P), NPBF)
    for gi, g in enumerate(GATES):
        WhT = Ws[g + "h"].T.astype(f32)  # [1024 in, 1024 out]
        Wh16 = (WhT * S).astype(NPF8)
        WxT = Ws[g + "x"].T.astype(f32)  # [300, 1024]
        Wx16 = (WxT * S).astype(NPF8)
        bg = bs[g].astype(f32)  # [1024]
        for m in range(MT):
            mc = slice(m * P, (m + 1) * P)
            j = gi * MT + m
            for k2 in range(4):
                b = ((gi * 4 + k2) * MT + m) * 256
                for i in range(2):
                    r = slice((2 * k2 + i) * P, (2 * k2 + i + 1) * P)
                    wh8[:, b + i * P : b + (i + 1) * P] = Wh16[r, mc]
            b = j * 256
            for i in range(2):
                r = slice(i * P, (i + 1) * P)
                wx8[:, b + i * P : b + (i + 1) * P] = Wx16[r, mc]
            p0 = 32 * (j % 3)
            b = (j // 3) * 256
            for i in range(2):
                wxb8[p0 : p0 + 22, b + i * P : b + (i + 1) * P] = \
                    Wx16[256 + i * 22 : 256 + (i + 1) * 22, mc]
            wxb8[p0 + 22, b : b + P] = (bg[mc] * S * S).astype(NPF8)
            for kx in range(2):
                bx = (j * 2 + kx) * P
                wxb[:, bx : bx + P] = WxT[kx * P : (kx + 1) * P, mc].astype(NPBF)
            p0 = 64 * (j % 2)
            bx = 2 * 32 * P + (j // 2) * P
            wxb[p0 : p0 + 44, bx : bx + P] = WxT[256:300, mc].astype(NPBF)
            wxb[p0 + 44, bx : bx + P] = bg[mc].astype(NPBF)
        for k in range(MT):
            whb[:, (gi * MT + k) * MEM : (gi * MT + k + 1) * MEM] = \
                WhT[k * P : (k + 1) * P, :].astype(NPBF)

    xT = embs.T.astype(f32)  # [300, 16383]

    def level_cols(lvl, k):
        nlv = 1 << lvl
        nl = nlv // NCORE
        return xT[:, nlv - 1 + k * nl : nlv - 1 + (k + 1) * nl]

    in_maps = []
    for k in range(NCORE):
        c8 = np.concatenate([level_cols(lvl, k) for lvl in FP8LVLS], axis=1)  # [300, 2016]
        c8s = c8 * S
        xt8a = np.zeros((P, 2 * NC8), NPF8)
        xt8b = np.zeros((P, 2 * NC8), NPF8)
        for i in range(2):
            xt8a[:, i * NC8 : (i + 1) * NC8] = c8s[i * P : (i + 1) * P].astype(NPF8)
            xt8b[0:22, i * NC8 : (i + 1) * NC8] = c8s[256 + i * 22 : 256 + (i + 1) * 22].astype(NPF8)
        xt8b[22, 0:NC8] = np.float32(1.0).astype(NPF8)  # const row (sub0 only)
        for s in range(1, 3):  # replicate for the 32-aligned B-chunk slots
            xt8b[32 * s : 32 * s + 23] = xt8b[0:23]
        cb = [level_cols(lvl, k) for lvl in BFLVLS]
        cb.append(xT[:, [3, 4, 5, 6, 1, 2, 0]])  # top-7 in heap level order
        cb = np.concatenate(cb, axis=1)  # [300, 38]
        xtb = np.zeros((P, 3 * NCB), NPBF)
        for kx in range(2):
            xtb[:, kx * NCB : (kx + 1) * NCB] = cb[kx * P : (kx + 1) * P].astype(NPBF)
        xtb[0:44, 2 * NCB : 3 * NCB] = cb[256:300].astype(NPBF)
        xtb[44, 2 * NCB : 3 * NCB] = np.float32(1.0).astype(NPBF)
        xtb[64 : 64 + 45, 2 * NCB : 3 * NCB] = xtb[0:45, 2 * NCB : 3 * NCB]
        in_maps.append({
            "xt8a": xt8a, "xt8b": xt8b, "xtb": xtb,
            "wx8": wx8, "wxb8": wxb8, "wh8": wh8, "whb": whb, "wxb": wxb,
        })
    return in_maps


def kernel(**inputs):
    embs = np.asarray(inputs["embs"], dtype=np.float32)
    depth = int(np.asarray(inputs["depth"]))
    assert depth == DEPTH and embs.shape == (2 ** DEPTH - 1, IN)
    Ws = {g + s: np.asarray(inputs["W" + g + s], dtype=np.float32)
          for g in GATES for s in "xh"}
    bs = {g: np.asarray(inputs["b" + g + "x"]) + np.asarray(inputs["b" + g + "h"])
          for g in GATES}

    nc = _get_program()
    in_maps = _prep_host_inputs(embs, Ws, bs)
    res = run_bass_kernel_spmd(nc, in_maps, core_ids=list(range(NCORE))).results
    root = res[0]["root"]  # [128, 16] f32: cols 0..7 = c chunks, 8..15 = h
    c_root = root[:, 0:8].T.reshape(MEM)
    h_root = root[:, 8:16].T.reshape(MEM)
    return np.stack([c_root, h_root]).astype(np.float32)
